# revision 65
# baseline (speedup 1.0000x reference)
"""Trainium2 Bass kernel for MixtureOfSoftmax attention.

Math (per batch b):
    pi    = softmax(W @ mean_q(Q))                      (n_mix,)
    S_m   = Q_m^T K_m / sqrt(dk)                        (Lq, Lk) per mixture
    attn  = sum_m pi_m * softmax_k(S_m)                 (Lq, Lk)
    out   = attn @ V^T                                  (Lq, dv)

Sharding: data-parallel over batch, one batch per NeuronCore (8 cores),
no collectives.  Each core runs the identical program on its own slice.

Per-core dataflow (ScalarE/exp-bound; modeled ~175us/core):
    Q,K loaded as float32r (full-rate matmul, no cast pass); V cast to
    fp16 and PE-transposed into (k, v) layout.
    pi:  column sums of Q (DVE) -> tiny matmul vs PE-transposed W ->
         exp+accum (ACT) -> reciprocal -> ones-matmul broadcast.
    For each of 16 q-tiles (128 rows):
      for m in 4 mixtures, k in 2 halves: QK matmul (C=64, f32r) ->
        PSUM fp32 (128,1024); ACT exp(scale=1/sqrt(dk)) PSUM->SBUF fp16
        with accum_out giving the softmax denominator half-sums.
      combined = sum_m (pi_m/Z_m) * E_m on DVE, incrementally per m
        (two-scalar tensor_scalar + chain adds) -> fp16 attn DMA
        (fp16 output is lossless here: the map is computed in fp16).
      PE 128x128 transposes of combined -> PSUM -> SBUF (groups of 4),
      AV matmul accumulating over 16 k-chunks -> out tile (fp32) -> DMA.
"""

import sys

import numpy as np

if "/opt/trn_rl_repo" not in sys.path:
    sys.path.insert(0, "/opt/trn_rl_repo")

import concourse.bacc as bacc
import concourse.tile as tile
from concourse import mybir
from concourse.bass_utils import run_bass_kernel_spmd
from concourse.masks import make_identity

F32 = mybir.dt.float32
BF16 = mybir.dt.bfloat16
F16 = mybir.dt.float16
AF = mybir.ActivationFunctionType
ALU = mybir.AluOpType
AX = mybir.AxisListType

P = 128


F32R = mybir.dt.float32r


def build(nc, LQ=2048, LK=2048, DK=256, DV=256, NM=4, qk_dtype="f32r",
          e_dtype="fp16", ebufs=3, cbufs=2, ctbufs=2, q0c=128,
          gk=4, gk2=8, zbufs=2, obufs=2):
    EDT = F16 if e_dtype == "fp16" else BF16
    ebufs, cbufs, ctbufs, q0c = (int(ebufs), int(cbufs), int(ctbufs),
                                 int(q0c))
    gk, gk2, zbufs, obufs = int(gk), int(gk2), int(zbufs), int(obufs)
    DC = DK // NM                      # channels per mixture (64)
    NQT = LQ // P                      # q tiles
    NKT = LK // P                      # k tiles (transpose/AV granularity)
    SH = LK // 2                       # score half-tile width (PSUM budget)
    CH = min(512, SH)                  # QK matmul chunk (fp32 psum bank limit)
    NCH = SH // CH
    inv_temp = 1.0 / float(np.sqrt(DK))

    q = nc.dram_tensor("q", (DK, LQ), F32, kind="ExternalInput")
    k = nc.dram_tensor("k", (DK, LK), F32, kind="ExternalInput")
    v = nc.dram_tensor("v", (DV, LK), F32, kind="ExternalInput")
    w = nc.dram_tensor("w", (NM, DK), F32, kind="ExternalInput")
    out = nc.dram_tensor("out", (LQ, DV), F32, kind="ExternalOutput")
    # attn values are fp16-representable by construction (the combined map
    # is computed in fp16), so the fp16 output tensor is lossless and halves
    # the DMA + host-transfer volume.
    attn = nc.dram_tensor("attn", (LQ, LK), EDT, kind="ExternalOutput")

    ndk = DK // P                      # number of 128-partition dk tiles (2)
    nv = DV // P                       # number of 128-partition dv tiles (2)

    with tile.TileContext(nc) as tc:
        with (
            tc.tile_pool(name="const", bufs=1) as const,
            tc.tile_pool(name="stage", bufs=2) as stage,
            tc.tile_pool(name="epool", bufs=ebufs) as epool,
            tc.tile_pool(name="cpool", bufs=cbufs) as cpool,
            tc.tile_pool(name="ctpool", bufs=ctbufs) as ctpool,
            tc.tile_pool(name="zpool", bufs=zbufs) as zpool,
            tc.tile_pool(name="obuf", bufs=obufs) as obuf,
            tc.tile_pool(name="spsum", bufs=2, space="PSUM") as spsum,
            tc.tile_pool(name="tpsum", bufs=1, space="PSUM") as tpsum,
            tc.tile_pool(name="opsum", bufs=2, space="PSUM") as opsum,
        ):
            # ---- constants
            ident_bf = const.tile([P, P], EDT, tag="ident_bf")
            make_identity(nc, ident_bf)
            ident_f = const.tile([P, P], F32, tag="ident_f")
            make_identity(nc, ident_f)

            vT = const.tile([P, NKT, DV], EDT, tag="vT")
            pibc = const.tile([P, NM], F32, tag="pibc")
            a = const.tile([P, ndk], F32, tag="avg")
            wT = const.tile([P, ndk, NM], F32, tag="wT")

            # ---- load Q (also column sums for pi), K
            if qk_dtype == "f32r":
                # keep fp32 bits; matmuls view them as float32r (full rate
                # at N>=256, better mantissa than bf16, no cast pass)
                qb = [const.tile([P, LQ], F32R, tag=f"qb{i}", name=f"qb{i}")
                      for i in range(ndk)]
                kb = [const.tile([P, LK], F32R, tag=f"kb{i}", name=f"kb{i}")
                      for i in range(ndk)]
                # chunked loads, first-needed first: qt0's q columns, then
                # all of k (exp half h consumes k columns [h*1024, ...])
                # tiny first q chunk: the first QK lhsT only needs qt0's
                # 128 columns, so land those before the bulk
                for i in range(ndk):
                    nc.sync.dma_start(out=qb[i][:, 0:q0c],
                                      in_=q[i * P:(i + 1) * P, 0:q0c].bitcast(F32R))
                    for c in range(4):
                        cs = slice(c * LK // 4, (c + 1) * LK // 4)
                        nc.sync.dma_start(out=kb[i][:, cs],
                                          in_=k[i * P:(i + 1) * P, cs].bitcast(F32R))
                for i in range(ndk):
                    bounds = sorted({q0c, LQ // 4, LQ // 2, 3 * LQ // 4, LQ})
                    lo = q0c
                    for hi in bounds:
                        if hi <= lo:
                            continue
                        cs = slice(lo, hi)
                        nc.sync.dma_start(out=qb[i][:, cs],
                                          in_=q[i * P:(i + 1) * P, cs].bitcast(F32R))
                        lo = hi
                for i in range(ndk):
                    nc.vector.reduce_sum(out=a[:, i:i + 1],
                                         in_=qb[i].bitcast(F32), axis=AX.X)
                qk_view = lambda ap: ap
            else:
                qb = [const.tile([P, LQ], EDT, tag=f"qb{i}", name=f"qb{i}")
                      for i in range(ndk)]
                kb = [const.tile([P, LK], EDT, tag=f"kb{i}", name=f"kb{i}")
                      for i in range(ndk)]
                for i in range(ndk):
                    qf = stage.tile([P, LQ], F32, tag="stage")
                    nc.sync.dma_start(out=qf, in_=q[i * P:(i + 1) * P, :])
                    nc.vector.tensor_copy(out=qb[i], in_=qf)
                    nc.vector.reduce_sum(out=a[:, i:i + 1], in_=qf, axis=AX.X)
                for i in range(ndk):
                    kf = stage.tile([P, LK], F32, tag="stage")
                    nc.sync.dma_start(out=kf, in_=k[i * P:(i + 1) * P, :])
                    nc.vector.tensor_copy(out=kb[i], in_=kf)
                qk_view = lambda ap: ap

            # ---- V: load, cast, transpose into (k, v) layout
            for i in range(nv):
                vf = stage.tile([P, LK], F32, tag="stage")
                nc.sync.dma_start(out=vf, in_=v[i * P:(i + 1) * P, :])
                vb = stage.tile([P, LK], EDT, tag="vb")
                nc.vector.tensor_copy(out=vb, in_=vf)
                pt = tpsum.tile([P, NKT * P], EDT, tag="pt")
                for kt in range(NKT):
                    nc.tensor.transpose(
                        pt[:, kt * P:(kt + 1) * P],
                        vb[:, kt * P:(kt + 1) * P],
                        ident_bf,
                    )
                nc.vector.tensor_copy(
                    out=vT[:, :, i * P:(i + 1) * P],
                    in_=pt.rearrange("p (kt vl) -> p kt vl", vl=P),
                )

            # ---- W^T via PE transpose (fp32: no DMA transpose)
            wsb = const.tile([NM, DK], F32, tag="wsb")
            nc.sync.dma_start(out=wsb, in_=w[:, :])
            for i in range(ndk):
                pw = opsum.tile([P, NM], F32, tag="po")
                nc.tensor.transpose(
                    pw, wsb[:, i * P:(i + 1) * P], ident_f[0:NM, 0:NM]
                )
                nc.vector.tensor_copy(out=wT[:, i, :], in_=pw)

            # ---- pi = softmax(W @ avg_q); logits psum holds LQ * logits
            ps_pi = opsum.tile([1, NM], F32, tag="po")
            for i in range(ndk):
                nc.tensor.matmul(
                    ps_pi, lhsT=a[:, i:i + 1], rhs=wT[:, i, :],
                    start=(i == 0), stop=(i == ndk - 1),
                )
            pi_s = zpool.tile([1, NM], F32, tag="pi_s")
            zp = zpool.tile([1, 1], F32, tag="zp")
            nc.scalar.activation(
                out=pi_s, in_=ps_pi, func=AF.Exp, scale=1.0 / LQ, accum_out=zp
            )
            rzp = zpool.tile([1, 1], F32, tag="rzp")
            nc.vector.reciprocal(out=rzp, in_=zp)
            pi1 = zpool.tile([1, NM], F32, tag="pi1")
            nc.vector.tensor_scalar(
                out=pi1, in0=pi_s, scalar1=rzp, scalar2=None, op0=ALU.mult
            )
            # broadcast pi to all 128 partitions: ones(128,1) @ pi1(1,4)
            ones1 = const.tile([1, P], F32, tag="ones1")
            nc.vector.memset(ones1, 1.0)
            ps_bc = opsum.tile([P, NM], F32, tag="po")
            nc.tensor.matmul(ps_bc, lhsT=ones1, rhs=pi1, start=True, stop=True)
            nc.vector.tensor_copy(out=pibc, in_=ps_bc)

            # ---- main loop over q tiles
            for qt in range(NQT):
                qsl = slice(qt * P, (qt + 1) * P)
                zt = zpool.tile([P, 2 * NM], F32, tag="zt")
                es = []
                for m in range(NM):
                    lo = (m % 2) * DC
                    e = epool.tile([P, LK], EDT, tag=f"e{m}", name=f"e{m}")
                    for h in range(2):
                        ps = spsum.tile([P, SH], F32, tag="ps")
                        for c in range(NCH):
                            kc = h * SH + c * CH
                            nc.tensor.matmul(
                                ps[:, c * CH:(c + 1) * CH],
                                lhsT=qk_view(qb[m // 2][lo:lo + DC, qsl]),
                                rhs=qk_view(kb[m // 2][lo:lo + DC, kc:kc + CH]),
                                start=True, stop=True,
                            )
                        nc.scalar.activation(
                            out=e[:, h * SH:(h + 1) * SH], in_=ps, func=AF.Exp,
                            scale=inv_temp, accum_out=zt[:, h * NM + m:h * NM + m + 1],
                        )
                    es.append(e)

                # combined = sum_m (pi_m / Z_m) * E_m   (bf16), incrementally:
                # each mixture's scale runs right after its own exps, the
                # running sum is a chain, so almost nothing serializes after
                # the last exp.
                last = qt == NQT - 1
                acc = None
                cbf = cpool.tile([P, LK], EDT, tag="cbf")
                for m in range(NM):
                    zs = zpool.tile([P, 1], F32, tag="zs", name="zs")
                    nc.vector.tensor_tensor(
                        out=zs, in0=zt[:, m:m + 1],
                        in1=zt[:, NM + m:NM + m + 1], op=ALU.add)
                    rz = zpool.tile([P, 1], F32, tag="rz", name="rz")
                    nc.vector.reciprocal(out=rz, in_=zs)
                    t = cpool.tile([P, LK], EDT, tag=f"t{m % 2}",
                                   name=f"t{m}")
                    if m < NM - 1:
                        nc.vector.tensor_scalar(
                            out=t, in0=es[m], scalar1=rz,
                            scalar2=pibc[:, m:m + 1],
                            op0=ALU.mult, op1=ALU.mult,
                        )
                        if m == 0:
                            acc = t
                        else:
                            nxt = cpool.tile([P, LK], EDT, tag=f"a{m}",
                                             name=f"a{m}")
                            nc.vector.tensor_tensor(out=nxt, in0=acc, in1=t,
                                                    op=ALU.add)
                            acc = nxt
                    else:
                        # last mixture: scale + final add + attn DMA per
                        # half so the transposes (which only need half of
                        # cbf each) start as early as possible
                        nhalf = 2 if last else 1
                        for h in range(nhalf):
                            hs = slice(h * LK // nhalf,
                                       (h + 1) * LK // nhalf)
                            nc.vector.tensor_scalar(
                                out=t[:, hs], in0=es[m][:, hs], scalar1=rz,
                                scalar2=pibc[:, m:m + 1],
                                op0=ALU.mult, op1=ALU.mult,
                            )
                            nc.vector.tensor_tensor(
                                out=cbf[:, hs], in0=acc[:, hs], in1=t[:, hs],
                                op=ALU.add)
                            nc.sync.dma_start(out=attn[qsl, hs],
                                              in_=cbf[:, hs])

                # transpose combined -> (k, q) blocks; evac + AV pipelined
                GK = min(gk, NKT)
                NG = NKT // GK
                pt = tpsum.tile([P, NKT * P], EDT, tag="pt")
                ct = ctpool.tile([P, NKT * P], EDT, tag="ct")
                po = opsum.tile([P, DV], F32, tag="po")
                gw = GK * P                     # columns per group
                pcs = [((gi % 2) * (NG // 2) + gi // 2) * gw if NG > 1 else 0
                       for gi in range(NG)]
                if last:
                    # tail: all transposes first so the in-order PE queue
                    # never blocks on an AV matmul whose ct evacuation
                    # hasn't happened yet; then evacs; then all AV matmuls
                    GK2 = min(gk2, NKT)
                    for kt in range(NKT):
                        nc.tensor.transpose(
                            pt[:, kt * P:(kt + 1) * P],
                            cbf[:, kt * P:(kt + 1) * P],
                            ident_bf,
                        )
                    for g in range(0, NKT, GK2):
                        nc.vector.tensor_copy(
                            out=ct[:, g * P:(g + GK2) * P],
                            in_=pt[:, g * P:(g + GK2) * P],
                        )
                    for kt in range(NKT):
                        nc.tensor.matmul(
                            po,
                            lhsT=ct[:, kt * P:(kt + 1) * P],
                            rhs=vT[:, kt, :],
                            start=(kt == 0), stop=(kt == NKT - 1),
                        )
                else:
                    # steady state: grouped interleave pipelines best with
                    # the next tile's QK matmuls behind it in the queue
                    for gi in range(NG):
                        g = gi * GK
                        for j in range(GK):
                            nc.tensor.transpose(
                                pt[:, pcs[gi] + j * P:pcs[gi] + (j + 1) * P],
                                cbf[:, (g + j) * P:(g + j + 1) * P],
                                ident_bf,
                            )
                        nc.vector.tensor_copy(
                            out=ct[:, g * P:(g + GK) * P],
                            in_=pt[:, pcs[gi]:pcs[gi] + gw],
                        )
                        for kt in range(g, g + GK):
                            nc.tensor.matmul(
                                po,
                                lhsT=ct[:, kt * P:(kt + 1) * P],
                                rhs=vT[:, kt, :],
                                start=(kt == 0), stop=(kt == NKT - 1),
                            )
                ob = obuf.tile([P, DV], F32, tag="ob")
                nc.vector.tensor_copy(out=ob, in_=po)
                nc.sync.dma_start(out=out[qsl, :], in_=ob)

    return nc


_CACHE = {}


def _get_nc(**kw):
    key = tuple(sorted(kw.items()))
    if key not in _CACHE:
        nc = bacc.Bacc()
        build(nc, **kw)
        nc.compile()
        _CACHE[key] = nc
    return _CACHE[key]


class _PjrtRunner:
    """run_bass_via_pjrt with the jitted executable built once and the
    donated output buffers created device-side (no host zero upload)."""

    def __init__(self, nc, n_cores):
        import jax
        import jax.numpy as jnp
        from jax.sharding import Mesh, NamedSharding, PartitionSpec
        from jax.experimental.shard_map import shard_map
        import concourse.mybir as mybir
        from concourse import bass2jax

        bass2jax.install_neuronx_cc_hook()
        self.jax = jax
        self.n_cores = n_cores
        partition_name = (nc.partition_id_tensor.name
                          if nc.partition_id_tensor else None)

        in_names, out_names, out_avals, zero_specs = [], [], [], []
        for alloc in nc.m.functions[0].allocations:
            if not isinstance(alloc, mybir.MemoryLocationSet):
                continue
            name = alloc.memorylocations[0].name
            if alloc.kind == "ExternalInput":
                if name != partition_name:
                    in_names.append(name)
            elif alloc.kind == "ExternalOutput":
                shape = tuple(alloc.tensor_shape)
                dtype = mybir.dt.np(alloc.dtype)
                out_names.append(name)
                out_avals.append(jax.core.ShapedArray(shape, dtype))
                zero_specs.append((shape, dtype))
        n_params = len(in_names)
        self.in_names = list(in_names)
        self.out_names = list(out_names)
        all_in_names = in_names + out_names
        if partition_name is not None:
            all_in_names.append(partition_name)

        def _body(*args):
            operands = list(args)
            if partition_name is not None:
                operands.append(bass2jax.partition_id_tensor())
            outs = bass2jax._bass_exec_p.bind(
                *operands,
                out_avals=tuple(out_avals),
                in_names=tuple(all_in_names),
                out_names=tuple(out_names),
                lowering_input_output_aliases=(),
                sim_require_finite=True,
                sim_require_nnan=True,
                nc=nc,
            )
            return tuple(outs)

        devices = jax.devices()[:n_cores]
        mesh = Mesh(np.asarray(devices), ("core",))
        spec = PartitionSpec("core")
        n_outs = len(out_names)
        self.fn = jax.jit(
            shard_map(
                _body, mesh=mesh,
                in_specs=(spec,) * (n_params + n_outs),
                out_specs=(spec,) * n_outs,
                check_rep=False,
            ),
            donate_argnums=tuple(range(n_params, n_params + n_outs)),
            keep_unused=True,
        )
        sharding = NamedSharding(mesh, spec)
        self.zeros_fn = jax.jit(
            lambda: tuple(
                jnp.zeros((n_cores * s[0],) + tuple(s[1:]), d)
                for s, d in zero_specs
            ),
            out_shardings=(sharding,) * n_outs,
        )

    def __call__(self, in_maps):
        globs = [
            np.concatenate([np.asarray(m[name]) for m in in_maps], axis=0)
            for name in self.in_names
        ]
        zeros = self.zeros_fn()
        outs = self.fn(*globs, *zeros)
        n = self.n_cores
        results = [dict() for _ in range(n)]
        for name, arr in zip(self.out_names, outs):
            arr = np.asarray(arr)
            per = arr.shape[0] // n
            for b in range(n):
                results[b][name] = arr[b * per:(b + 1) * per]
        return results


def _get_runner(B):
    key = ("runner", B)
    if key not in _CACHE:
        _CACHE[key] = _PjrtRunner(_get_nc(), B)
    return _CACHE[key]


def kernel(query, key, value, weights):
    query = np.ascontiguousarray(np.asarray(query, dtype=np.float32))
    key_ = np.ascontiguousarray(np.asarray(key, dtype=np.float32))
    value = np.ascontiguousarray(np.asarray(value, dtype=np.float32))
    weights = np.ascontiguousarray(np.asarray(weights, dtype=np.float32))

    B = query.shape[0]
    runner = _get_runner(B)
    in_maps = [
        {"q": query[b], "k": key_[b], "v": value[b], "w": weights}
        for b in range(B)
    ]
    results = runner(in_maps)
    out = np.stack([results[b]["out"] for b in range(B)])
    attn = np.stack([results[b]["attn"] for b in range(B)]).astype(np.float32)
    return out, attn


def kernel_via_spmd(query, key, value, weights):
    """Reference path through run_bass_kernel_spmd (for cross-checking)."""
    query = np.ascontiguousarray(np.asarray(query, dtype=np.float32))
    key_ = np.ascontiguousarray(np.asarray(key, dtype=np.float32))
    value = np.ascontiguousarray(np.asarray(value, dtype=np.float32))
    weights = np.ascontiguousarray(np.asarray(weights, dtype=np.float32))
    B = query.shape[0]
    nc = _get_nc()
    in_maps = [
        {"q": query[b], "k": key_[b], "v": value[b], "w": weights}
        for b in range(B)
    ]
    res = run_bass_kernel_spmd(nc, in_maps, core_ids=list(range(B)))
    out = np.stack([res.results[b]["out"] for b in range(B)])
    attn = np.stack([res.results[b]["attn"]
                     for b in range(B)]).astype(np.float32)
    return out, attn


# revision 66
# speedup vs baseline: 1.0044x; 1.0044x over previous
"""Trainium2 Bass kernel for MixtureOfSoftmax attention.

Math (per batch b):
    pi    = softmax(W @ mean_q(Q))                      (n_mix,)
    S_m   = Q_m^T K_m / sqrt(dk)                        (Lq, Lk) per mixture
    attn  = sum_m pi_m * softmax_k(S_m)                 (Lq, Lk)
    out   = attn @ V^T                                  (Lq, dv)

Sharding: data-parallel over batch, one batch per NeuronCore (8 cores),
no collectives.  Each core runs the identical program on its own slice.

Per-core dataflow (ScalarE/exp-bound; modeled ~175us/core):
    Q,K loaded as float32r (full-rate matmul, no cast pass); V cast to
    fp16 and PE-transposed into (k, v) layout.
    pi:  column sums of Q (DVE) -> tiny matmul vs PE-transposed W ->
         exp+accum (ACT) -> reciprocal -> ones-matmul broadcast.
    For each of 16 q-tiles (128 rows):
      for m in 4 mixtures, k in 2 halves: QK matmul (C=64, f32r) ->
        PSUM fp32 (128,1024); ACT exp(scale=1/sqrt(dk)) PSUM->SBUF fp16
        with accum_out giving the softmax denominator half-sums.
      combined = sum_m (pi_m/Z_m) * E_m on DVE, incrementally per m
        (two-scalar tensor_scalar + chain adds) -> fp16 attn DMA
        (fp16 output is lossless here: the map is computed in fp16).
      PE 128x128 transposes of combined -> PSUM -> SBUF (groups of 4),
      AV matmul accumulating over 16 k-chunks -> out tile (fp32) -> DMA.
"""

import sys

import numpy as np

if "/opt/trn_rl_repo" not in sys.path:
    sys.path.insert(0, "/opt/trn_rl_repo")

import concourse.bacc as bacc
import concourse.tile as tile
from concourse import mybir
from concourse.bass_utils import run_bass_kernel_spmd
from concourse.masks import make_identity

F32 = mybir.dt.float32
BF16 = mybir.dt.bfloat16
F16 = mybir.dt.float16
AF = mybir.ActivationFunctionType
ALU = mybir.AluOpType
AX = mybir.AxisListType

P = 128


F32R = mybir.dt.float32r


def build(nc, LQ=2048, LK=2048, DK=256, DV=256, NM=4, qk_dtype="f32r",
          e_dtype="fp16", ebufs=3, cbufs=2, ctbufs=2, q0c=128,
          gk=4, gk2=8, zbufs=2, obufs=2):
    EDT = F16 if e_dtype == "fp16" else BF16
    ebufs, cbufs, ctbufs, q0c = (int(ebufs), int(cbufs), int(ctbufs),
                                 int(q0c))
    gk, gk2, zbufs, obufs = int(gk), int(gk2), int(zbufs), int(obufs)
    DC = DK // NM                      # channels per mixture (64)
    NQT = LQ // P                      # q tiles
    NKT = LK // P                      # k tiles (transpose/AV granularity)
    SH = LK // 2                       # score half-tile width (PSUM budget)
    CH = min(512, SH)                  # QK matmul chunk (fp32 psum bank limit)
    NCH = SH // CH
    inv_temp = 1.0 / float(np.sqrt(DK))

    q = nc.dram_tensor("q", (DK, LQ), F32, kind="ExternalInput")
    k = nc.dram_tensor("k", (DK, LK), F32, kind="ExternalInput")
    v = nc.dram_tensor("v", (DV, LK), F32, kind="ExternalInput")
    w = nc.dram_tensor("w", (NM, DK), F32, kind="ExternalInput")
    out = nc.dram_tensor("out", (LQ, DV), F32, kind="ExternalOutput")
    # attn values are fp16-representable by construction (the combined map
    # is computed in fp16), so the fp16 output tensor is lossless and halves
    # the DMA + host-transfer volume.
    attn = nc.dram_tensor("attn", (LQ, LK), EDT, kind="ExternalOutput")

    ndk = DK // P                      # number of 128-partition dk tiles (2)
    nv = DV // P                       # number of 128-partition dv tiles (2)

    with tile.TileContext(nc) as tc:
        with (
            tc.tile_pool(name="const", bufs=1) as const,
            tc.tile_pool(name="stage", bufs=2) as stage,
            tc.tile_pool(name="epool", bufs=ebufs) as epool,
            tc.tile_pool(name="cpool", bufs=cbufs) as cpool,
            tc.tile_pool(name="ctpool", bufs=ctbufs) as ctpool,
            tc.tile_pool(name="zpool", bufs=zbufs) as zpool,
            tc.tile_pool(name="obuf", bufs=obufs) as obuf,
            tc.tile_pool(name="spsum", bufs=2, space="PSUM") as spsum,
            tc.tile_pool(name="tpsum", bufs=1, space="PSUM") as tpsum,
            tc.tile_pool(name="opsum", bufs=2, space="PSUM") as opsum,
        ):
            # ---- constants
            ident_bf = const.tile([P, P], EDT, tag="ident_bf")
            make_identity(nc, ident_bf)
            ident_f = const.tile([P, P], F32, tag="ident_f")
            make_identity(nc, ident_f)

            # PE warmup: dummy transposes while the first DMAs are in
            # flight, so the HAM clock ramp (~3.4us) completes before the
            # first QK matmul instead of throttling it.  The tiny copy is
            # a consumer that keeps dead-code elimination away.
            warm = tpsum.tile([P, P], EDT, tag="pt", name="warm")
            for _ in range(24):
                nc.tensor.transpose(warm, ident_bf, ident_bf)
            wsink = const.tile([P, 1], EDT, tag="wsink")
            nc.vector.tensor_copy(out=wsink, in_=warm[:, 0:1])

            vT = const.tile([P, NKT, DV], EDT, tag="vT")
            pibc = const.tile([P, NM], F32, tag="pibc")
            a = const.tile([P, ndk], F32, tag="avg")
            wT = const.tile([P, ndk, NM], F32, tag="wT")

            # ---- load Q (also column sums for pi), K
            if qk_dtype == "f32r":
                # keep fp32 bits; matmuls view them as float32r (full rate
                # at N>=256, better mantissa than bf16, no cast pass)
                qb = [const.tile([P, LQ], F32R, tag=f"qb{i}", name=f"qb{i}")
                      for i in range(ndk)]
                kb = [const.tile([P, LK], F32R, tag=f"kb{i}", name=f"kb{i}")
                      for i in range(ndk)]
                # chunked loads, first-needed first: qt0's q columns, then
                # all of k (exp half h consumes k columns [h*1024, ...])
                # tiny first q chunk: the first QK lhsT only needs qt0's
                # 128 columns, so land those before the bulk
                for i in range(ndk):
                    nc.sync.dma_start(out=qb[i][:, 0:q0c],
                                      in_=q[i * P:(i + 1) * P, 0:q0c].bitcast(F32R))
                    for c in range(4):
                        cs = slice(c * LK // 4, (c + 1) * LK // 4)
                        nc.sync.dma_start(out=kb[i][:, cs],
                                          in_=k[i * P:(i + 1) * P, cs].bitcast(F32R))
                for i in range(ndk):
                    bounds = sorted({q0c, LQ // 4, LQ // 2, 3 * LQ // 4, LQ})
                    lo = q0c
                    for hi in bounds:
                        if hi <= lo:
                            continue
                        cs = slice(lo, hi)
                        nc.sync.dma_start(out=qb[i][:, cs],
                                          in_=q[i * P:(i + 1) * P, cs].bitcast(F32R))
                        lo = hi
                for i in range(ndk):
                    nc.vector.reduce_sum(out=a[:, i:i + 1],
                                         in_=qb[i].bitcast(F32), axis=AX.X)
                qk_view = lambda ap: ap
            else:
                qb = [const.tile([P, LQ], EDT, tag=f"qb{i}", name=f"qb{i}")
                      for i in range(ndk)]
                kb = [const.tile([P, LK], EDT, tag=f"kb{i}", name=f"kb{i}")
                      for i in range(ndk)]
                for i in range(ndk):
                    qf = stage.tile([P, LQ], F32, tag="stage")
                    nc.sync.dma_start(out=qf, in_=q[i * P:(i + 1) * P, :])
                    nc.vector.tensor_copy(out=qb[i], in_=qf)
                    nc.vector.reduce_sum(out=a[:, i:i + 1], in_=qf, axis=AX.X)
                for i in range(ndk):
                    kf = stage.tile([P, LK], F32, tag="stage")
                    nc.sync.dma_start(out=kf, in_=k[i * P:(i + 1) * P, :])
                    nc.vector.tensor_copy(out=kb[i], in_=kf)
                qk_view = lambda ap: ap

            # ---- V: load, cast, transpose into (k, v) layout
            for i in range(nv):
                vf = stage.tile([P, LK], F32, tag="stage")
                nc.sync.dma_start(out=vf, in_=v[i * P:(i + 1) * P, :])
                vb = stage.tile([P, LK], EDT, tag="vb")
                nc.vector.tensor_copy(out=vb, in_=vf)
                pt = tpsum.tile([P, NKT * P], EDT, tag="pt")
                for kt in range(NKT):
                    nc.tensor.transpose(
                        pt[:, kt * P:(kt + 1) * P],
                        vb[:, kt * P:(kt + 1) * P],
                        ident_bf,
                    )
                nc.vector.tensor_copy(
                    out=vT[:, :, i * P:(i + 1) * P],
                    in_=pt.rearrange("p (kt vl) -> p kt vl", vl=P),
                )

            # ---- W^T via PE transpose (fp32: no DMA transpose)
            wsb = const.tile([NM, DK], F32, tag="wsb")
            nc.sync.dma_start(out=wsb, in_=w[:, :])
            for i in range(ndk):
                pw = opsum.tile([P, NM], F32, tag="po")
                nc.tensor.transpose(
                    pw, wsb[:, i * P:(i + 1) * P], ident_f[0:NM, 0:NM]
                )
                nc.vector.tensor_copy(out=wT[:, i, :], in_=pw)

            # ---- pi = softmax(W @ avg_q); logits psum holds LQ * logits
            ps_pi = opsum.tile([1, NM], F32, tag="po")
            for i in range(ndk):
                nc.tensor.matmul(
                    ps_pi, lhsT=a[:, i:i + 1], rhs=wT[:, i, :],
                    start=(i == 0), stop=(i == ndk - 1),
                )
            pi_s = zpool.tile([1, NM], F32, tag="pi_s")
            zp = zpool.tile([1, 1], F32, tag="zp")
            nc.scalar.activation(
                out=pi_s, in_=ps_pi, func=AF.Exp, scale=1.0 / LQ, accum_out=zp
            )
            rzp = zpool.tile([1, 1], F32, tag="rzp")
            nc.vector.reciprocal(out=rzp, in_=zp)
            pi1 = zpool.tile([1, NM], F32, tag="pi1")
            nc.vector.tensor_scalar(
                out=pi1, in0=pi_s, scalar1=rzp, scalar2=None, op0=ALU.mult
            )
            # broadcast pi to all 128 partitions: ones(128,1) @ pi1(1,4)
            ones1 = const.tile([1, P], F32, tag="ones1")
            nc.vector.memset(ones1, 1.0)
            ps_bc = opsum.tile([P, NM], F32, tag="po")
            nc.tensor.matmul(ps_bc, lhsT=ones1, rhs=pi1, start=True, stop=True)
            nc.vector.tensor_copy(out=pibc, in_=ps_bc)

            # ---- main loop over q tiles
            for qt in range(NQT):
                qsl = slice(qt * P, (qt + 1) * P)
                zt = zpool.tile([P, 2 * NM], F32, tag="zt")
                es = []
                for m in range(NM):
                    lo = (m % 2) * DC
                    e = epool.tile([P, LK], EDT, tag=f"e{m}", name=f"e{m}")
                    for h in range(2):
                        ps = spsum.tile([P, SH], F32, tag="ps")
                        for c in range(NCH):
                            kc = h * SH + c * CH
                            nc.tensor.matmul(
                                ps[:, c * CH:(c + 1) * CH],
                                lhsT=qk_view(qb[m // 2][lo:lo + DC, qsl]),
                                rhs=qk_view(kb[m // 2][lo:lo + DC, kc:kc + CH]),
                                start=True, stop=True,
                            )
                        nc.scalar.activation(
                            out=e[:, h * SH:(h + 1) * SH], in_=ps, func=AF.Exp,
                            scale=inv_temp, accum_out=zt[:, h * NM + m:h * NM + m + 1],
                        )
                    es.append(e)

                # combined = sum_m (pi_m / Z_m) * E_m   (bf16), incrementally:
                # each mixture's scale runs right after its own exps, the
                # running sum is a chain, so almost nothing serializes after
                # the last exp.
                last = qt == NQT - 1
                acc = None
                cbf = cpool.tile([P, LK], EDT, tag="cbf")
                for m in range(NM):
                    zs = zpool.tile([P, 1], F32, tag="zs", name="zs")
                    nc.vector.tensor_tensor(
                        out=zs, in0=zt[:, m:m + 1],
                        in1=zt[:, NM + m:NM + m + 1], op=ALU.add)
                    rz = zpool.tile([P, 1], F32, tag="rz", name="rz")
                    nc.vector.reciprocal(out=rz, in_=zs)
                    t = cpool.tile([P, LK], EDT, tag=f"t{m % 2}",
                                   name=f"t{m}")
                    if m < NM - 1:
                        nc.vector.tensor_scalar(
                            out=t, in0=es[m], scalar1=rz,
                            scalar2=pibc[:, m:m + 1],
                            op0=ALU.mult, op1=ALU.mult,
                        )
                        if m == 0:
                            acc = t
                        else:
                            nxt = cpool.tile([P, LK], EDT, tag=f"a{m}",
                                             name=f"a{m}")
                            nc.vector.tensor_tensor(out=nxt, in0=acc, in1=t,
                                                    op=ALU.add)
                            acc = nxt
                    else:
                        # last mixture: scale + final add + attn DMA per
                        # half so the transposes (which only need half of
                        # cbf each) start as early as possible
                        nhalf = 2 if last else 1
                        for h in range(nhalf):
                            hs = slice(h * LK // nhalf,
                                       (h + 1) * LK // nhalf)
                            nc.vector.tensor_scalar(
                                out=t[:, hs], in0=es[m][:, hs], scalar1=rz,
                                scalar2=pibc[:, m:m + 1],
                                op0=ALU.mult, op1=ALU.mult,
                            )
                            nc.vector.tensor_tensor(
                                out=cbf[:, hs], in0=acc[:, hs], in1=t[:, hs],
                                op=ALU.add)
                            nc.sync.dma_start(out=attn[qsl, hs],
                                              in_=cbf[:, hs])

                # transpose combined -> (k, q) blocks; evac + AV pipelined
                GK = min(gk, NKT)
                NG = NKT // GK
                pt = tpsum.tile([P, NKT * P], EDT, tag="pt")
                ct = ctpool.tile([P, NKT * P], EDT, tag="ct")
                po = opsum.tile([P, DV], F32, tag="po")
                gw = GK * P                     # columns per group
                pcs = [((gi % 2) * (NG // 2) + gi // 2) * gw if NG > 1 else 0
                       for gi in range(NG)]
                if last:
                    # tail: all transposes first so the in-order PE queue
                    # never blocks on an AV matmul whose ct evacuation
                    # hasn't happened yet; then evacs; then all AV matmuls
                    GK2 = min(gk2, NKT)
                    for kt in range(NKT):
                        nc.tensor.transpose(
                            pt[:, kt * P:(kt + 1) * P],
                            cbf[:, kt * P:(kt + 1) * P],
                            ident_bf,
                        )
                    for g in range(0, NKT, GK2):
                        nc.vector.tensor_copy(
                            out=ct[:, g * P:(g + GK2) * P],
                            in_=pt[:, g * P:(g + GK2) * P],
                        )
                    for kt in range(NKT):
                        nc.tensor.matmul(
                            po,
                            lhsT=ct[:, kt * P:(kt + 1) * P],
                            rhs=vT[:, kt, :],
                            start=(kt == 0), stop=(kt == NKT - 1),
                        )
                else:
                    # steady state: grouped interleave pipelines best with
                    # the next tile's QK matmuls behind it in the queue
                    for gi in range(NG):
                        g = gi * GK
                        for j in range(GK):
                            nc.tensor.transpose(
                                pt[:, pcs[gi] + j * P:pcs[gi] + (j + 1) * P],
                                cbf[:, (g + j) * P:(g + j + 1) * P],
                                ident_bf,
                            )
                        nc.vector.tensor_copy(
                            out=ct[:, g * P:(g + GK) * P],
                            in_=pt[:, pcs[gi]:pcs[gi] + gw],
                        )
                        for kt in range(g, g + GK):
                            nc.tensor.matmul(
                                po,
                                lhsT=ct[:, kt * P:(kt + 1) * P],
                                rhs=vT[:, kt, :],
                                start=(kt == 0), stop=(kt == NKT - 1),
                            )
                ob = obuf.tile([P, DV], F32, tag="ob")
                nc.vector.tensor_copy(out=ob, in_=po)
                nc.sync.dma_start(out=out[qsl, :], in_=ob)

    return nc


_CACHE = {}


def _get_nc(**kw):
    key = tuple(sorted(kw.items()))
    if key not in _CACHE:
        nc = bacc.Bacc()
        build(nc, **kw)
        nc.compile()
        _CACHE[key] = nc
    return _CACHE[key]


class _PjrtRunner:
    """run_bass_via_pjrt with the jitted executable built once and the
    donated output buffers created device-side (no host zero upload)."""

    def __init__(self, nc, n_cores):
        import jax
        import jax.numpy as jnp
        from jax.sharding import Mesh, NamedSharding, PartitionSpec
        from jax.experimental.shard_map import shard_map
        import concourse.mybir as mybir
        from concourse import bass2jax

        bass2jax.install_neuronx_cc_hook()
        self.jax = jax
        self.n_cores = n_cores
        partition_name = (nc.partition_id_tensor.name
                          if nc.partition_id_tensor else None)

        in_names, out_names, out_avals, zero_specs = [], [], [], []
        for alloc in nc.m.functions[0].allocations:
            if not isinstance(alloc, mybir.MemoryLocationSet):
                continue
            name = alloc.memorylocations[0].name
            if alloc.kind == "ExternalInput":
                if name != partition_name:
                    in_names.append(name)
            elif alloc.kind == "ExternalOutput":
                shape = tuple(alloc.tensor_shape)
                dtype = mybir.dt.np(alloc.dtype)
                out_names.append(name)
                out_avals.append(jax.core.ShapedArray(shape, dtype))
                zero_specs.append((shape, dtype))
        n_params = len(in_names)
        self.in_names = list(in_names)
        self.out_names = list(out_names)
        all_in_names = in_names + out_names
        if partition_name is not None:
            all_in_names.append(partition_name)

        def _body(*args):
            operands = list(args)
            if partition_name is not None:
                operands.append(bass2jax.partition_id_tensor())
            outs = bass2jax._bass_exec_p.bind(
                *operands,
                out_avals=tuple(out_avals),
                in_names=tuple(all_in_names),
                out_names=tuple(out_names),
                lowering_input_output_aliases=(),
                sim_require_finite=True,
                sim_require_nnan=True,
                nc=nc,
            )
            return tuple(outs)

        devices = jax.devices()[:n_cores]
        mesh = Mesh(np.asarray(devices), ("core",))
        spec = PartitionSpec("core")
        n_outs = len(out_names)
        self.fn = jax.jit(
            shard_map(
                _body, mesh=mesh,
                in_specs=(spec,) * (n_params + n_outs),
                out_specs=(spec,) * n_outs,
                check_rep=False,
            ),
            donate_argnums=tuple(range(n_params, n_params + n_outs)),
            keep_unused=True,
        )
        sharding = NamedSharding(mesh, spec)
        self.zeros_fn = jax.jit(
            lambda: tuple(
                jnp.zeros((n_cores * s[0],) + tuple(s[1:]), d)
                for s, d in zero_specs
            ),
            out_shardings=(sharding,) * n_outs,
        )

    def __call__(self, in_maps):
        globs = [
            np.concatenate([np.asarray(m[name]) for m in in_maps], axis=0)
            for name in self.in_names
        ]
        zeros = self.zeros_fn()
        outs = self.fn(*globs, *zeros)
        n = self.n_cores
        results = [dict() for _ in range(n)]
        for name, arr in zip(self.out_names, outs):
            arr = np.asarray(arr)
            per = arr.shape[0] // n
            for b in range(n):
                results[b][name] = arr[b * per:(b + 1) * per]
        return results


def _get_runner(B):
    key = ("runner", B)
    if key not in _CACHE:
        _CACHE[key] = _PjrtRunner(_get_nc(), B)
    return _CACHE[key]


def kernel(query, key, value, weights):
    query = np.ascontiguousarray(np.asarray(query, dtype=np.float32))
    key_ = np.ascontiguousarray(np.asarray(key, dtype=np.float32))
    value = np.ascontiguousarray(np.asarray(value, dtype=np.float32))
    weights = np.ascontiguousarray(np.asarray(weights, dtype=np.float32))

    B = query.shape[0]
    runner = _get_runner(B)
    in_maps = [
        {"q": query[b], "k": key_[b], "v": value[b], "w": weights}
        for b in range(B)
    ]
    results = runner(in_maps)
    out = np.stack([results[b]["out"] for b in range(B)])
    attn = np.stack([results[b]["attn"] for b in range(B)]).astype(np.float32)
    return out, attn


def kernel_via_spmd(query, key, value, weights):
    """Reference path through run_bass_kernel_spmd (for cross-checking)."""
    query = np.ascontiguousarray(np.asarray(query, dtype=np.float32))
    key_ = np.ascontiguousarray(np.asarray(key, dtype=np.float32))
    value = np.ascontiguousarray(np.asarray(value, dtype=np.float32))
    weights = np.ascontiguousarray(np.asarray(weights, dtype=np.float32))
    B = query.shape[0]
    nc = _get_nc()
    in_maps = [
        {"q": query[b], "k": key_[b], "v": value[b], "w": weights}
        for b in range(B)
    ]
    res = run_bass_kernel_spmd(nc, in_maps, core_ids=list(range(B)))
    out = np.stack([res.results[b]["out"] for b in range(B)])
    attn = np.stack([res.results[b]["attn"]
                     for b in range(B)]).astype(np.float32)
    return out, attn


# revision 72
# speedup vs baseline: 1.0079x; 1.0035x over previous
"""Trainium2 Bass kernel for MixtureOfSoftmax attention.

Math (per batch b):
    pi    = softmax(W @ mean_q(Q))                      (n_mix,)
    S_m   = Q_m^T K_m / sqrt(dk)                        (Lq, Lk) per mixture
    attn  = sum_m pi_m * softmax_k(S_m)                 (Lq, Lk)
    out   = attn @ V^T                                  (Lq, dv)

Sharding: data-parallel over batch, one batch per NeuronCore (8 cores),
no collectives.  Each core runs the identical program on its own slice.

Per-core dataflow (ScalarE/exp-bound; modeled ~175us/core):
    Q,K loaded as float32r (full-rate matmul, no cast pass); V cast to
    fp16 and PE-transposed into (k, v) layout.
    pi:  column sums of Q (DVE) -> tiny matmul vs PE-transposed W ->
         exp+accum (ACT) -> reciprocal -> ones-matmul broadcast.
    For each of 16 q-tiles (128 rows):
      for m in 4 mixtures, k in 2 halves: QK matmul (C=64, f32r) ->
        PSUM fp32 (128,1024); ACT exp(scale=1/sqrt(dk)) PSUM->SBUF fp16
        with accum_out giving the softmax denominator half-sums.
      combined = sum_m (pi_m/Z_m) * E_m on DVE, incrementally per m
        (two-scalar tensor_scalar + chain adds) -> fp16 attn DMA
        (fp16 output is lossless here: the map is computed in fp16).
      PE 128x128 transposes of combined -> PSUM -> SBUF (groups of 4),
      AV matmul accumulating over 16 k-chunks -> out tile (fp32) -> DMA.
"""

import sys

import numpy as np

if "/opt/trn_rl_repo" not in sys.path:
    sys.path.insert(0, "/opt/trn_rl_repo")

import concourse.bacc as bacc
import concourse.tile as tile
from concourse import mybir
from concourse.bass_utils import run_bass_kernel_spmd
from concourse.masks import make_identity

F32 = mybir.dt.float32
BF16 = mybir.dt.bfloat16
F16 = mybir.dt.float16
AF = mybir.ActivationFunctionType
ALU = mybir.AluOpType
AX = mybir.AxisListType

P = 128


F32R = mybir.dt.float32r


def build(nc, LQ=2048, LK=2048, DK=256, DV=256, NM=4, qk_dtype="f32r",
          e_dtype="fp16", ebufs=3, cbufs=2, ctbufs=2, q0c=128,
          gk=4, gk2=8, zbufs=2, obufs=2):
    EDT = F16 if e_dtype == "fp16" else BF16
    ebufs, cbufs, ctbufs, q0c = (int(ebufs), int(cbufs), int(ctbufs),
                                 int(q0c))
    gk, gk2, zbufs, obufs = int(gk), int(gk2), int(zbufs), int(obufs)
    DC = DK // NM                      # channels per mixture (64)
    NQT = LQ // P                      # q tiles
    NKT = LK // P                      # k tiles (transpose/AV granularity)
    SH = LK // 2                       # score half-tile width (PSUM budget)
    CH = min(512, SH)                  # QK matmul chunk (fp32 psum bank limit)
    NCH = SH // CH
    inv_temp = 1.0 / float(np.sqrt(DK))

    q = nc.dram_tensor("q", (DK, LQ), F32, kind="ExternalInput")
    k = nc.dram_tensor("k", (DK, LK), F32, kind="ExternalInput")
    v = nc.dram_tensor("v", (DV, LK), F32, kind="ExternalInput")
    w = nc.dram_tensor("w", (NM, DK), F32, kind="ExternalInput")
    out = nc.dram_tensor("out", (LQ, DV), F32, kind="ExternalOutput")
    # attn values are fp16-representable by construction (the combined map
    # is computed in fp16), so the fp16 output tensor is lossless and halves
    # the DMA + host-transfer volume.
    attn = nc.dram_tensor("attn", (LQ, LK), EDT, kind="ExternalOutput")

    ndk = DK // P                      # number of 128-partition dk tiles (2)
    nv = DV // P                       # number of 128-partition dv tiles (2)

    with tile.TileContext(nc) as tc:
        with (
            tc.tile_pool(name="const", bufs=1) as const,
            tc.tile_pool(name="stage", bufs=2) as stage,
            tc.tile_pool(name="epool", bufs=ebufs) as epool,
            tc.tile_pool(name="cpool", bufs=cbufs) as cpool,
            tc.tile_pool(name="ctpool", bufs=ctbufs) as ctpool,
            tc.tile_pool(name="zpool", bufs=zbufs) as zpool,
            tc.tile_pool(name="obuf", bufs=obufs) as obuf,
            tc.tile_pool(name="spsum", bufs=2, space="PSUM") as spsum,
            tc.tile_pool(name="tpsum", bufs=1, space="PSUM") as tpsum,
            tc.tile_pool(name="opsum", bufs=2, space="PSUM") as opsum,
        ):
            # ---- constants
            ident_bf = const.tile([P, P], EDT, tag="ident_bf")
            make_identity(nc, ident_bf)
            ident_f = const.tile([P, P], F32, tag="ident_f")
            make_identity(nc, ident_f)

            # PE warmup: dummy transposes while the first DMAs are in
            # flight, so the HAM clock ramp (~3.4us) completes before the
            # first QK matmul instead of throttling it.  The tiny copy is
            # a consumer that keeps dead-code elimination away.
            warm = tpsum.tile([P, P], EDT, tag="pt", name="warm")
            for _ in range(24):
                nc.tensor.transpose(warm, ident_bf, ident_bf)
            wsink = const.tile([P, 1], EDT, tag="wsink")
            nc.vector.tensor_copy(out=wsink, in_=warm[:, 0:1])

            vT = const.tile([P, NKT, DV], EDT, tag="vT")
            pibc = const.tile([P, NM], F32, tag="pibc")
            a = const.tile([P, ndk], F32, tag="avg")
            wT = const.tile([P, ndk, NM], F32, tag="wT")

            # ---- load Q (also column sums for pi), K
            if qk_dtype == "f32r":
                # keep fp32 bits; matmuls view them as float32r (full rate
                # at N>=256, better mantissa than bf16, no cast pass)
                qb = [const.tile([P, LQ], F32R, tag=f"qb{i}", name=f"qb{i}")
                      for i in range(ndk)]
                kb = [const.tile([P, LK], F32R, tag=f"kb{i}", name=f"kb{i}")
                      for i in range(ndk)]
                # chunked loads, first-needed first: qt0's q columns, then
                # all of k (exp half h consumes k columns [h*1024, ...])
                # tiny first q chunk: the first QK lhsT only needs qt0's
                # 128 columns, so land those before the bulk
                for i in range(ndk):
                    nc.sync.dma_start(out=qb[i][:, 0:q0c],
                                      in_=q[i * P:(i + 1) * P, 0:q0c].bitcast(F32R))
                    for c in range(4):
                        cs = slice(c * LK // 4, (c + 1) * LK // 4)
                        nc.sync.dma_start(out=kb[i][:, cs],
                                          in_=k[i * P:(i + 1) * P, cs].bitcast(F32R))
                for i in range(ndk):
                    bounds = sorted({q0c, LQ // 4, LQ // 2, 3 * LQ // 4, LQ})
                    lo = q0c
                    for hi in bounds:
                        if hi <= lo:
                            continue
                        cs = slice(lo, hi)
                        nc.sync.dma_start(out=qb[i][:, cs],
                                          in_=q[i * P:(i + 1) * P, cs].bitcast(F32R))
                        lo = hi
                for i in range(ndk):
                    nc.vector.reduce_sum(out=a[:, i:i + 1],
                                         in_=qb[i].bitcast(F32), axis=AX.X)
                qk_view = lambda ap: ap
            else:
                qb = [const.tile([P, LQ], EDT, tag=f"qb{i}", name=f"qb{i}")
                      for i in range(ndk)]
                kb = [const.tile([P, LK], EDT, tag=f"kb{i}", name=f"kb{i}")
                      for i in range(ndk)]
                for i in range(ndk):
                    qf = stage.tile([P, LQ], F32, tag="stage")
                    nc.sync.dma_start(out=qf, in_=q[i * P:(i + 1) * P, :])
                    nc.vector.tensor_copy(out=qb[i], in_=qf)
                    nc.vector.reduce_sum(out=a[:, i:i + 1], in_=qf, axis=AX.X)
                for i in range(ndk):
                    kf = stage.tile([P, LK], F32, tag="stage")
                    nc.sync.dma_start(out=kf, in_=k[i * P:(i + 1) * P, :])
                    nc.vector.tensor_copy(out=kb[i], in_=kf)
                qk_view = lambda ap: ap

            # ---- V: load, cast, transpose into (k, v) layout
            for i in range(nv):
                vf = stage.tile([P, LK], F32, tag="stage")
                nc.sync.dma_start(out=vf, in_=v[i * P:(i + 1) * P, :])
                vb = stage.tile([P, LK], EDT, tag="vb")
                nc.vector.tensor_copy(out=vb, in_=vf)
                pt = tpsum.tile([P, NKT * P], EDT, tag="pt")
                for kt in range(NKT):
                    nc.tensor.transpose(
                        pt[:, kt * P:(kt + 1) * P],
                        vb[:, kt * P:(kt + 1) * P],
                        ident_bf,
                    )
                nc.vector.tensor_copy(
                    out=vT[:, :, i * P:(i + 1) * P],
                    in_=pt.rearrange("p (kt vl) -> p kt vl", vl=P),
                )

            # ---- W^T via PE transpose (fp32: no DMA transpose)
            wsb = const.tile([NM, DK], F32, tag="wsb")
            nc.sync.dma_start(out=wsb, in_=w[:, :])
            for i in range(ndk):
                pw = opsum.tile([P, NM], F32, tag="po")
                nc.tensor.transpose(
                    pw, wsb[:, i * P:(i + 1) * P], ident_f[0:NM, 0:NM]
                )
                nc.vector.tensor_copy(out=wT[:, i, :], in_=pw)

            # ---- pi = softmax(W @ avg_q); logits psum holds LQ * logits
            ps_pi = opsum.tile([1, NM], F32, tag="po")
            for i in range(ndk):
                nc.tensor.matmul(
                    ps_pi, lhsT=a[:, i:i + 1], rhs=wT[:, i, :],
                    start=(i == 0), stop=(i == ndk - 1),
                )
            pi_s = zpool.tile([1, NM], F32, tag="pi_s")
            zp = zpool.tile([1, 1], F32, tag="zp")
            nc.scalar.activation(
                out=pi_s, in_=ps_pi, func=AF.Exp, scale=1.0 / LQ, accum_out=zp
            )
            rzp = zpool.tile([1, 1], F32, tag="rzp")
            nc.vector.reciprocal(out=rzp, in_=zp)
            pi1 = zpool.tile([1, NM], F32, tag="pi1")
            nc.vector.tensor_scalar(
                out=pi1, in0=pi_s, scalar1=rzp, scalar2=None, op0=ALU.mult
            )
            # broadcast pi to all 128 partitions: ones(128,1) @ pi1(1,4)
            ones1 = const.tile([1, P], F32, tag="ones1")
            nc.vector.memset(ones1, 1.0)
            ps_bc = opsum.tile([P, NM], F32, tag="po")
            nc.tensor.matmul(ps_bc, lhsT=ones1, rhs=pi1, start=True, stop=True)
            nc.vector.tensor_copy(out=pibc, in_=ps_bc)

            # ---- main loop over q tiles
            for qt in range(NQT):
                qsl = slice(qt * P, (qt + 1) * P)
                zt = zpool.tile([P, 2 * NM], F32, tag="zt")
                es = []
                for m in range(NM):
                    lo = (m % 2) * DC
                    e = epool.tile([P, LK], EDT, tag=f"e{m}", name=f"e{m}")
                    for h in range(2):
                        ps = spsum.tile([P, SH], F32, tag="ps")
                        for c in range(NCH):
                            kc = h * SH + c * CH
                            nc.tensor.matmul(
                                ps[:, c * CH:(c + 1) * CH],
                                lhsT=qk_view(qb[m // 2][lo:lo + DC, qsl]),
                                rhs=qk_view(kb[m // 2][lo:lo + DC, kc:kc + CH]),
                                start=True, stop=True,
                            )
                        nc.scalar.activation(
                            out=e[:, h * SH:(h + 1) * SH], in_=ps, func=AF.Exp,
                            scale=inv_temp, accum_out=zt[:, h * NM + m:h * NM + m + 1],
                        )
                    es.append(e)

                # combined = sum_m (pi_m / Z_m) * E_m   (bf16), incrementally:
                # each mixture's scale runs right after its own exps, the
                # running sum is a chain, so almost nothing serializes after
                # the last exp.
                last = qt == NQT - 1
                acc = None
                cbf = cpool.tile([P, LK], EDT, tag="cbf")
                for m in range(NM):
                    zs = zpool.tile([P, 1], F32, tag="zs", name="zs")
                    nc.vector.tensor_tensor(
                        out=zs, in0=zt[:, m:m + 1],
                        in1=zt[:, NM + m:NM + m + 1], op=ALU.add)
                    rz = zpool.tile([P, 1], F32, tag="rz", name="rz")
                    nc.vector.reciprocal(out=rz, in_=zs)
                    t = cpool.tile([P, LK], EDT, tag=f"t{m % 2}",
                                   name=f"t{m}")
                    if m < NM - 1:
                        nc.vector.tensor_scalar(
                            out=t, in0=es[m], scalar1=rz,
                            scalar2=pibc[:, m:m + 1],
                            op0=ALU.mult, op1=ALU.mult,
                        )
                        if m == 0:
                            acc = t
                        else:
                            nxt = cpool.tile([P, LK], EDT, tag=f"a{m}",
                                             name=f"a{m}")
                            nc.vector.tensor_tensor(out=nxt, in0=acc, in1=t,
                                                    op=ALU.add)
                            acc = nxt
                    else:
                        # last mixture: scale + final add + attn DMA per
                        # half so the transposes (which only need half of
                        # cbf each) start as early as possible
                        nhalf = 2 if last else 1
                        for h in range(nhalf):
                            hs = slice(h * LK // nhalf,
                                       (h + 1) * LK // nhalf)
                            nc.vector.tensor_scalar(
                                out=t[:, hs], in0=es[m][:, hs], scalar1=rz,
                                scalar2=pibc[:, m:m + 1],
                                op0=ALU.mult, op1=ALU.mult,
                            )
                            nc.vector.tensor_tensor(
                                out=cbf[:, hs], in0=acc[:, hs], in1=t[:, hs],
                                op=ALU.add)
                            nc.sync.dma_start(out=attn[qsl, hs],
                                              in_=cbf[:, hs])

                # transpose combined -> (k, q) blocks; evac + AV pipelined
                GK = min(gk, NKT)
                NG = NKT // GK
                ct = ctpool.tile([P, NKT * P], EDT, tag="ct")
                gw = GK * P                     # columns per group
                pcs = [((gi % 2) * (NG // 2) + gi // 2) * gw if NG > 1 else 0
                       for gi in range(NG)]
                if last:
                    # tail: all transposes first so the in-order PE queue
                    # never blocks on an AV matmul whose ct evacuation
                    # hasn't happened yet.  The two transpose halves go to
                    # SEPARATE one-bank tiles (borrowed from the po tag's
                    # slots) so each evacuation depends only on its own
                    # half's transposes — deps are tensor-granular.
                    HK = max(NKT // 2, 1)
                    pth = [opsum.tile([P, HK * P], EDT, tag="po",
                                      name=f"pth{j}") for j in range(2)]
                    for kt in range(NKT):
                        nc.tensor.transpose(
                            pth[kt // HK][:, (kt % HK) * P:(kt % HK + 1) * P],
                            cbf[:, kt * P:(kt + 1) * P],
                            ident_bf,
                        )
                    for j in range(2):
                        nc.vector.tensor_copy(
                            out=ct[:, j * HK * P:(j + 1) * HK * P],
                            in_=pth[j],
                        )
                    po = opsum.tile([P, DV], F32, tag="po")
                    for kt in range(NKT):
                        nc.tensor.matmul(
                            po,
                            lhsT=ct[:, kt * P:(kt + 1) * P],
                            rhs=vT[:, kt, :],
                            start=(kt == 0), stop=(kt == NKT - 1),
                        )
                else:
                    # steady state: grouped interleave pipelines best with
                    # the next tile's QK matmuls behind it in the queue
                    pt = tpsum.tile([P, NKT * P], EDT, tag="pt")
                    po = opsum.tile([P, DV], F32, tag="po")
                    for gi in range(NG):
                        g = gi * GK
                        for j in range(GK):
                            nc.tensor.transpose(
                                pt[:, pcs[gi] + j * P:pcs[gi] + (j + 1) * P],
                                cbf[:, (g + j) * P:(g + j + 1) * P],
                                ident_bf,
                            )
                        nc.vector.tensor_copy(
                            out=ct[:, g * P:(g + GK) * P],
                            in_=pt[:, pcs[gi]:pcs[gi] + gw],
                        )
                        for kt in range(g, g + GK):
                            nc.tensor.matmul(
                                po,
                                lhsT=ct[:, kt * P:(kt + 1) * P],
                                rhs=vT[:, kt, :],
                                start=(kt == 0), stop=(kt == NKT - 1),
                            )
                ob = obuf.tile([P, DV], F32, tag="ob")
                nc.vector.tensor_copy(out=ob, in_=po)
                nc.sync.dma_start(out=out[qsl, :], in_=ob)

    return nc


_CACHE = {}


def _get_nc(**kw):
    key = tuple(sorted(kw.items()))
    if key not in _CACHE:
        nc = bacc.Bacc()
        build(nc, **kw)
        nc.compile()
        _CACHE[key] = nc
    return _CACHE[key]


class _PjrtRunner:
    """run_bass_via_pjrt with the jitted executable built once and the
    donated output buffers created device-side (no host zero upload)."""

    def __init__(self, nc, n_cores):
        import jax
        import jax.numpy as jnp
        from jax.sharding import Mesh, NamedSharding, PartitionSpec
        from jax.experimental.shard_map import shard_map
        import concourse.mybir as mybir
        from concourse import bass2jax

        bass2jax.install_neuronx_cc_hook()
        self.jax = jax
        self.n_cores = n_cores
        partition_name = (nc.partition_id_tensor.name
                          if nc.partition_id_tensor else None)

        in_names, out_names, out_avals, zero_specs = [], [], [], []
        for alloc in nc.m.functions[0].allocations:
            if not isinstance(alloc, mybir.MemoryLocationSet):
                continue
            name = alloc.memorylocations[0].name
            if alloc.kind == "ExternalInput":
                if name != partition_name:
                    in_names.append(name)
            elif alloc.kind == "ExternalOutput":
                shape = tuple(alloc.tensor_shape)
                dtype = mybir.dt.np(alloc.dtype)
                out_names.append(name)
                out_avals.append(jax.core.ShapedArray(shape, dtype))
                zero_specs.append((shape, dtype))
        n_params = len(in_names)
        self.in_names = list(in_names)
        self.out_names = list(out_names)
        all_in_names = in_names + out_names
        if partition_name is not None:
            all_in_names.append(partition_name)

        def _body(*args):
            operands = list(args)
            if partition_name is not None:
                operands.append(bass2jax.partition_id_tensor())
            outs = bass2jax._bass_exec_p.bind(
                *operands,
                out_avals=tuple(out_avals),
                in_names=tuple(all_in_names),
                out_names=tuple(out_names),
                lowering_input_output_aliases=(),
                sim_require_finite=True,
                sim_require_nnan=True,
                nc=nc,
            )
            return tuple(outs)

        devices = jax.devices()[:n_cores]
        mesh = Mesh(np.asarray(devices), ("core",))
        spec = PartitionSpec("core")
        n_outs = len(out_names)
        self.fn = jax.jit(
            shard_map(
                _body, mesh=mesh,
                in_specs=(spec,) * (n_params + n_outs),
                out_specs=(spec,) * n_outs,
                check_rep=False,
            ),
            donate_argnums=tuple(range(n_params, n_params + n_outs)),
            keep_unused=True,
        )
        sharding = NamedSharding(mesh, spec)
        self.zeros_fn = jax.jit(
            lambda: tuple(
                jnp.zeros((n_cores * s[0],) + tuple(s[1:]), d)
                for s, d in zero_specs
            ),
            out_shardings=(sharding,) * n_outs,
        )

    def __call__(self, in_maps):
        globs = [
            np.concatenate([np.asarray(m[name]) for m in in_maps], axis=0)
            for name in self.in_names
        ]
        zeros = self.zeros_fn()
        outs = self.fn(*globs, *zeros)
        n = self.n_cores
        results = [dict() for _ in range(n)]
        for name, arr in zip(self.out_names, outs):
            arr = np.asarray(arr)
            per = arr.shape[0] // n
            for b in range(n):
                results[b][name] = arr[b * per:(b + 1) * per]
        return results


def _get_runner(B):
    key = ("runner", B)
    if key not in _CACHE:
        _CACHE[key] = _PjrtRunner(_get_nc(), B)
    return _CACHE[key]


def kernel(query, key, value, weights):
    query = np.ascontiguousarray(np.asarray(query, dtype=np.float32))
    key_ = np.ascontiguousarray(np.asarray(key, dtype=np.float32))
    value = np.ascontiguousarray(np.asarray(value, dtype=np.float32))
    weights = np.ascontiguousarray(np.asarray(weights, dtype=np.float32))

    B = query.shape[0]
    runner = _get_runner(B)
    in_maps = [
        {"q": query[b], "k": key_[b], "v": value[b], "w": weights}
        for b in range(B)
    ]
    results = runner(in_maps)
    out = np.stack([results[b]["out"] for b in range(B)])
    attn = np.stack([results[b]["attn"] for b in range(B)]).astype(np.float32)
    return out, attn


def kernel_via_spmd(query, key, value, weights):
    """Reference path through run_bass_kernel_spmd (for cross-checking)."""
    query = np.ascontiguousarray(np.asarray(query, dtype=np.float32))
    key_ = np.ascontiguousarray(np.asarray(key, dtype=np.float32))
    value = np.ascontiguousarray(np.asarray(value, dtype=np.float32))
    weights = np.ascontiguousarray(np.asarray(weights, dtype=np.float32))
    B = query.shape[0]
    nc = _get_nc()
    in_maps = [
        {"q": query[b], "k": key_[b], "v": value[b], "w": weights}
        for b in range(B)
    ]
    res = run_bass_kernel_spmd(nc, in_maps, core_ids=list(range(B)))
    out = np.stack([res.results[b]["out"] for b in range(B)])
    attn = np.stack([res.results[b]["attn"]
                     for b in range(B)]).astype(np.float32)
    return out, attn
